# revision 1
# baseline (speedup 1.0000x reference)
"""Multi-head attention (lazy K/V projections) Trainium2 Bass kernel.

Problem: nn_MultiHeadAttention_54520314856024
  B=8, SQ=SK=1024, D=1024, E=128, H=32
  keys  = einsum('bsd,hde->hbse', states, Wk) + bk
  vals  = einsum('bsd,hde->hbse', states, Wv) + bv
  attn  = softmax(einsum('bqe,hbke->hbqk', query, keys) / sqrt(E))
  ctx   = einsum('hbqk,hbke->hbqe', attn, vals) -> concat heads -> @ Wc + bc

Sharding: batch-parallel, one batch element per NeuronCore (8 cores).

Design notes:
  - bk dropped on device: softmax over k is invariant to the per-q additive
    shift (bk . q), so the keys bias cancels exactly.
  - bv folded into bc on host: sum_k attn = 1 implies ctx = ctx0 + bv, so
    out = ctx0 @ Wc + (bc + bv.flatten() @ Wc). Removes all vals-bias work.
  - scores/keys/vals/transposes share one 3-buffer single-bank PSUM pool
    (their phases are disjoint); denominator + final-projection share a
    2-buffer pool; ctx accumulators a 2-buffer pool. 7 PSUM banks total.
  - per head, the kt loop emits scores+exp for kt and denominator/ctx
    matmuls for kt-1 (software pipelining) so PE never waits on ACT's exp.
  - the final projection of head h is deferred until after head h+1's keys
    so DVE recip/mul latency hides under PE keys matmuls.
"""

import sys

for _p in ("/opt/trn_rl_repo",):
    if _p not in sys.path:
        sys.path.insert(0, _p)

import numpy as np

import concourse.bass as bass
import concourse.mybir as mybir
import concourse.tile as tile
from concourse import bacc, bass_utils
from concourse.masks import make_identity

B, SQ, SK = 8, 1024, 1024
D, E, H = 1024, 128, 32
P = 128          # partition width
DCH = D // P     # 8 d-chunks
KT = SK // P     # 8 k-tiles
G = 4            # heads per vals-group
NG = H // G      # 8 groups
NHALF = 512      # matmul moving-dim chunk (fp32 max)
SCALE = 1.0 / float(np.sqrt(E))

F32 = mybir.dt.float32
F32R = mybir.dt.float32r

N_CORES = 8

_COMPILED = {}
_ONES_SQ = np.ones((P, P), np.float32)


def build_nc(mm_dtype="f32r", repeat=1):
    """Build the single-core Bass program (SPMD across 8 cores).

    repeat > 1 re-emits the whole computation that many times (identical
    work each pass) for launch-overhead-amortized timing; the final DRAM
    output is written by every pass (all identical).
    """
    MT = F32R if mm_dtype == "f32r" else F32

    nc = bacc.Bacc("TRN2", target_bir_lowering=False, debug=False)

    statesT = nc.dram_tensor("statesT", [D, SK], MT, kind="ExternalInput").ap()
    queryT = nc.dram_tensor("queryT", [E, SQ], MT, kind="ExternalInput").ap()
    WkT = nc.dram_tensor("WkT", [NG, D, G * E], MT, kind="ExternalInput").ap()
    WvT = nc.dram_tensor("WvT", [NG, D, G * E], MT, kind="ExternalInput").ap()
    Wc = nc.dram_tensor("Wc", [H * E, E], MT, kind="ExternalInput").ap()
    bcT = nc.dram_tensor("bcT", [E, 1], F32, kind="ExternalInput").ap()
    onesSQ = nc.dram_tensor("onesSQ", [P, P], MT, kind="ExternalInput").ap()
    out = nc.dram_tensor("out", [SQ, E], F32, kind="ExternalOutput").ap()

    Wc3 = Wc.rearrange("(h e) f -> h e f", e=P)

    from contextlib import ExitStack

    with tile.TileContext(nc) as tc, ExitStack() as es:
        constp = es.enter_context(tc.tile_pool(name="const", bufs=1))
        statesp = es.enter_context(tc.tile_pool(name="states", bufs=DCH))
        queryp = es.enter_context(tc.tile_pool(name="query", bufs=1))
        wkp = es.enter_context(tc.tile_pool(name="wk", bufs=10))
        wvp = es.enter_context(tc.tile_pool(name="wv", bufs=10))
        wcp = es.enter_context(tc.tile_pool(name="wc", bufs=4))
        keysp = es.enter_context(tc.tile_pool(name="keys", bufs=2))
        expp = es.enter_context(tc.tile_pool(name="exps", bufs=8))
        valsp = es.enter_context(tc.tile_pool(name="vals", bufs=12))
        recipp = es.enter_context(tc.tile_pool(name="recip", bufs=2))
        ctxp = es.enter_context(tc.tile_pool(name="ctx", bufs=2))
        finalp = es.enter_context(tc.tile_pool(name="final", bufs=1))
        outp = es.enter_context(tc.tile_pool(name="outs", bufs=4))
        dsump = es.enter_context(tc.tile_pool(name="dsum", bufs=12))
        ps_main = es.enter_context(tc.tile_pool(name="ps_main", bufs=4, space="PSUM"))
        ps_denom = es.enter_context(tc.tile_pool(name="ps_denom", bufs=2, space="PSUM"))
        ps_ctx = es.enter_context(tc.tile_pool(name="ps_ctx", bufs=2, space="PSUM"))

        # ---- constants ----
        ones_sq = constp.tile([P, P], MT)
        nc.sync.dma_start(ones_sq[:], onesSQ[:])
        ident = constp.tile([P, P], F32)
        make_identity(nc, ident[:])
        bc_t = constp.tile([E, 1], F32)
        nc.sync.dma_start(bc_t[:], bcT[:])

        # ---- resident activations ----
        st = []
        for d in range(DCH):
            st_t = statesp.tile([P, SK], MT, name="st_t")
            nc.sync.dma_start(st_t[:], statesT[d * P : (d + 1) * P, :])
            st.append(st_t)
        q_t = queryp.tile([E, SQ], MT)
        nc.sync.dma_start(q_t[:], queryT[:])

        final_t = finalp.tile([E, SQ], F32)

        def emit_final(rep, h, ctx_sb):
            """Final projection contribution of head h into final_t."""
            wc_t = wcp.tile([P, P], MT, name="wc_t")
            nc.sync.dma_start(wc_t[:], Wc3[h])
            for qh in range(2):
                pf = ps_denom.tile(
                    [P, NHALF], F32, tag="denom", name="pf"
                )
                nc.tensor.matmul(
                    pf[:],
                    (wc_t[:]),
                    (ctx_sb[:, qh * NHALF : (qh + 1) * NHALF]),
                    start=True,
                    stop=True,
                )
                if h == 0:
                    nc.vector.tensor_scalar(
                        final_t[:, qh * NHALF : (qh + 1) * NHALF],
                        pf[:],
                        bc_t[:],
                        None,
                        op0=mybir.AluOpType.add,
                    )
                else:
                    nc.vector.tensor_add(
                        final_t[:, qh * NHALF : (qh + 1) * NHALF],
                        final_t[:, qh * NHALF : (qh + 1) * NHALF],
                        pf[:],
                    )

        pending_final = None

        for rep in range(repeat):
            def emit_wdma(gw):
                wvt, wkt = [], []
                for d in range(DCH):
                    wv_t = wvp.tile([P, G * E], MT, name="wv_t")
                    nc.sync.dma_start(wv_t[:], WvT[gw, d * P : (d + 1) * P, :])
                    wvt.append(wv_t)
                for d in range(DCH):
                    wk_t = wkp.tile([P, G * E], MT, name="wk_t")
                    nc.sync.dma_start(wk_t[:], WkT[gw, d * P : (d + 1) * P, :])
                    wkt.append(wk_t)
                return wvt, wkt

            def emit_vals_tile(kt, wvt):
                pv = ps_main.tile([P, G * E], F32, tag="m", name="pv")
                for d in range(DCH):
                    nc.tensor.matmul(
                        pv[:],
                        (st[d][:, kt * P : (kt + 1) * P]),
                        (wvt[d][:]),
                        start=(d == 0),
                        stop=(d == DCH - 1),
                    )
                v_sb = valsp.tile([P, G * E], MT, name="v_sb")
                nc.vector.tensor_copy(v_sb[:], pv[:])
                return v_sb

            next_w = emit_wdma(0)
            next_vals = []

            for g in range(NG):
                wv_tiles, wk_tiles = next_w
                # vals tiles not already built during the previous group's
                # last head:
                vals_tiles = next_vals
                for kt in range(len(vals_tiles), KT):
                    vals_tiles.append(emit_vals_tile(kt, wv_tiles))
                next_vals = []

                def emit_keys_block(hg_k):
                    """Full 16-matmul keys emission for head (g, hg_k)."""
                    ksb = keysp.tile([E, SK], MT, name="keys_sb")
                    for half in range(2):
                        pk = ps_main.tile(
                            [P, NHALF], F32, tag="m", name="pk"
                        )
                        for d in range(DCH):
                            nc.tensor.matmul(
                                pk[:],
                                (wk_tiles[d][:, hg_k * E : (hg_k + 1) * E]),
                                (st[d][:, half * NHALF : (half + 1) * NHALF]),
                                start=(d == 0),
                                stop=(d == DCH - 1),
                            )
                        nc.vector.tensor_copy(
                            ksb[:, half * NHALF : (half + 1) * NHALF], pk[:]
                        )
                    return ksb

                # first head's keys as one block (nothing to hide them under)
                next_keys_sb = emit_keys_block(0)

                for hg in range(G):
                    h = g * G + hg
                    keys_sb = next_keys_sb

                    # previous head's final projection: scheduled into the
                    # early kt-phase, after DVE recip/mul of head h-1 is done.
                    if pending_final is not None:
                        emit_final(*pending_final)
                        pending_final = None

                    # next head's keys are interleaved into this head's kt
                    # loop (2 matmuls per kt) to keep PE saturated while the
                    # softmax phase is ACT-paced. kts 0-3 build half 0,
                    # kts 4-7 half 1.
                    ik_hg = hg + 1 if hg + 1 < G else None
                    if ik_hg is not None:
                        next_keys_sb = keysp.tile([E, SK], MT, name="keys_sb")
                        ik_pk = None
                    iv_w = None
                    if ik_hg is None and g + 1 < NG:
                        next_w = emit_wdma(g + 1)
                        iv_w = next_w[0]
                        iv_pv = None

                    # ---- scores -> exp -> denom/ctx accumulate, per (kt, qh) ----
                    pd = [
                        ps_denom.tile(
                            [P, NHALF], F32, tag="denom", name="pd"
                        )
                        for i in range(2)
                    ]
                    pc = [
                        ps_ctx.tile(
                            [P, NHALF], F32, tag="ctx", name="pc"
                        )
                        for i in range(2)
                    ]
                    # Software-pipelined: emit kt's scores+exp, then kt-1's
                    # ctx matmuls (so PE never waits on the just-issued exp).
                    # Denominator: gpsimd pair-adds of exp tiles as pairs
                    # complete (Pool engine is otherwise idle), DVE finishes
                    # the reduction tree, then ONE ones-matmul per half.
                    pending = None  # ([ex_qh0, ex_qh1], kt)
                    last_ex = [None, None]
                    partials = [[], []]
                    for kt in range(KT):
                        exs = []
                        for qh in range(2):
                            ps = ps_main.tile(
                                [P, NHALF], F32, tag="m",
                                name="ps",
                            )
                            nc.tensor.matmul(
                                ps[:],
                                (keys_sb[:, kt * P : (kt + 1) * P]),
                                (q_t[:, qh * NHALF : (qh + 1) * NHALF]),
                                start=True,
                                stop=True,
                            )
                            ex = expp.tile(
                                [P, NHALF], MT, name="ex"
                            )
                            nc.scalar.activation(
                                ex[:], ps[:], mybir.ActivationFunctionType.Exp,
                                scale=SCALE,
                            )
                            exs.append(ex)
                            if kt % 2 == 1:
                                gsum = dsump.tile([P, NHALF], MT, name="gsum")
                                nc.gpsimd.tensor_add(
                                    gsum[:], last_ex[qh][:], ex[:]
                                )
                                partials[qh].append(gsum)
                            else:
                                last_ex[qh] = ex
                        if iv_w is not None:
                            # 2 vals matmuls of group g+1 per kt: tile kt//4,
                            # d-chunks 2*(kt%4) and 2*(kt%4)+1
                            vkt = kt // 4
                            loc = kt % 4
                            if loc == 0:
                                iv_pv = ps_main.tile(
                                    [P, G * E], F32, tag="m", name="pv"
                                )
                            for d in (2 * loc, 2 * loc + 1):
                                nc.tensor.matmul(
                                    iv_pv[:],
                                    (st[d][:, vkt * P : (vkt + 1) * P]),
                                    (iv_w[d][:]),
                                    start=(d == 0),
                                    stop=(d == DCH - 1),
                                )
                            if loc == 3:
                                v_sb = valsp.tile([P, G * E], MT, name="v_sb")
                                nc.vector.tensor_copy(v_sb[:], iv_pv[:])
                                next_vals.append(v_sb)
                        if ik_hg is not None:
                            half = kt // 4
                            loc = kt % 4
                            if loc == 0:
                                ik_pk = ps_main.tile(
                                    [P, NHALF], F32, tag="m", name="pk"
                                )
                            for d in (2 * loc, 2 * loc + 1):
                                nc.tensor.matmul(
                                    ik_pk[:],
                                    (wk_tiles[d][:, ik_hg * E : (ik_hg + 1) * E]),
                                    (st[d][:, half * NHALF : (half + 1) * NHALF]),
                                    start=(d == 0),
                                    stop=(d == DCH - 1),
                                )
                            if loc == 3:
                                nc.vector.tensor_copy(
                                    next_keys_sb[
                                        :, half * NHALF : (half + 1) * NHALF
                                    ],
                                    ik_pk[:],
                                )
                        if pending is not None:
                            pexs, pkt = pending
                            for qh in range(2):
                                nc.tensor.matmul(
                                    pc[qh][:],
                                    (vals_tiles[pkt][:, hg * E : (hg + 1) * E]),
                                    (pexs[qh][:]),
                                    start=(pkt == 0),
                                    stop=False,
                                )
                        pending = (exs, kt)
                    pexs, pkt = pending
                    for qh in range(2):
                        nc.tensor.matmul(
                            pc[qh][:],
                            (vals_tiles[pkt][:, hg * E : (hg + 1) * E]),
                            (pexs[qh][:]),
                            start=False,
                            stop=True,
                        )
                    for qh in range(2):
                        p0, p1, p2, p3 = partials[qh]
                        d1 = dsump.tile([P, NHALF], MT, name="gsum")
                        nc.vector.tensor_add(d1[:], p0[:], p1[:])
                        d2 = dsump.tile([P, NHALF], MT, name="gsum")
                        nc.vector.tensor_add(d2[:], p2[:], p3[:])
                        ssum = dsump.tile([P, NHALF], MT, name="gsum")
                        nc.vector.tensor_add(ssum[:], d1[:], d2[:])
                        nc.tensor.matmul(
                            pd[qh][:], (ones_sq[:]), (ssum[:]),
                            start=True, stop=True,
                        )

                    # ---- normalize: ctx_sb = pc / denom ----
                    ctx_sb = ctxp.tile([E, SQ], MT, name="ctx_sb")
                    for qh in range(2):
                        rec = recipp.tile(
                            [P, NHALF], F32, name="rec"
                        )
                        nc.vector.reciprocal_approx_fast(out=rec[:], in_=pd[qh][:])
                        nc.vector.tensor_mul(
                            ctx_sb[:, qh * NHALF : (qh + 1) * NHALF],
                            pc[qh][:],
                            rec[:],
                        )

                    # defer the final projection until after the next head's keys
                    pending_final = (rep, h, ctx_sb)

            emit_final(*pending_final)
            pending_final = None

            # ---- transpose finalT -> out [SQ, E] ----
            for qt in range(KT):
                pt = ps_main.tile([P, P], F32, tag="m", name="pt")
                nc.tensor.transpose(
                    pt[:], final_t[:, qt * P : (qt + 1) * P], ident[:]
                )
                o_sb = outp.tile([P, E], F32, name="o_sb")
                nc.vector.tensor_copy(o_sb[:], pt[:])
                nc.sync.dma_start(out[qt * P : (qt + 1) * P, :], o_sb[:])

    nc.compile()
    return nc


def _prep_inputs(query, states, Wk, bk, Wv, bv, Wc, bc):
    """Host-side sharding: per-core input maps (core c == batch element c).

    bk is dropped (softmax shift invariance); bv is folded into bc:
    out = ctx0 @ Wc + (bc + bv.flatten() @ Wc).
    """
    query = np.asarray(query, np.float32)
    states = np.asarray(states, np.float32)
    Wk = np.asarray(Wk, np.float32)
    Wv = np.asarray(Wv, np.float32)
    Wc = np.asarray(Wc, np.float32)
    bv = np.asarray(bv, np.float32)
    bc = np.asarray(bc, np.float32)

    WkT = np.ascontiguousarray(
        Wk.transpose(1, 0, 2).reshape(D, NG, G * E).transpose(1, 0, 2)
    )
    WvT = np.ascontiguousarray(
        Wv.transpose(1, 0, 2).reshape(D, NG, G * E).transpose(1, 0, 2)
    )
    bc_eff = (
        np.asarray(bc, np.float64)
        + np.asarray(bv, np.float64).reshape(H * E) @ np.asarray(Wc, np.float64)
    ).astype(np.float32)
    bcT = np.ascontiguousarray(bc_eff.reshape(E, 1))
    WcC = np.ascontiguousarray(Wc)

    in_maps = []
    for c in range(N_CORES):
        in_maps.append(
            {
                "statesT": np.ascontiguousarray(states[c].T),  # [D, SK]
                "queryT": np.ascontiguousarray(query[c].T),    # [E, SQ]
                "WkT": WkT,
                "WvT": WvT,
                "Wc": WcC,
                "bcT": bcT,
                "onesSQ": _ONES_SQ,
            }
        )
    return in_maps


def get_nc(mm_dtype="f32r", repeat=1):
    key = (mm_dtype, repeat)
    nc = _COMPILED.get(key)
    if nc is None:
        nc = build_nc(mm_dtype, repeat=repeat)
        _COMPILED[key] = nc
    return nc


def kernel(query, states, Wk, bk, Wv, bv, Wc, bc):
    nc = get_nc()
    in_maps = _prep_inputs(query, states, Wk, bk, Wv, bv, Wc, bc)
    res = bass_utils.run_bass_kernel_spmd(nc, in_maps, list(range(N_CORES)))
    return np.stack([res.results[c]["out"] for c in range(N_CORES)], axis=0)



# revision 2
# speedup vs baseline: 1.0878x; 1.0878x over previous
"""Multi-head attention (lazy K/V projections) Trainium2 Bass kernel.

Problem: nn_MultiHeadAttention_54520314856024
  B=8, SQ=SK=1024, D=1024, E=128, H=32
  keys  = einsum('bsd,hde->hbse', states, Wk) + bk
  vals  = einsum('bsd,hde->hbse', states, Wv) + bv
  attn  = softmax(einsum('bqe,hbke->hbqk', query, keys) / sqrt(E))
  ctx   = einsum('hbqk,hbke->hbqe', attn, vals) -> concat heads -> @ Wc + bc
  This is out = sum_h softmax(q keys_h^T) (states Wv_h Wc_h) + bc_eff.

Sharding: batch-parallel, one batch element per NeuronCore (8 cores).

Design notes:
  - fp16 matmul inputs: same 1 cycle/row PE rate as f32r but enables FWL
    (fast weight load) so LDWEIGHTS hides under matmuls, and 2x DVE/GpSimd
    elementwise rates + half the DMA bytes. fp16 (10-bit mantissa) over
    bf16 for accuracy margin; all tensors here are O(1) so no range risk.
  - Wc folded into Wv on host: vc_h = states @ (Wv_h Wc_h), so the per-head
    normalized context IS the head's output contribution; the final [H*E,E]
    projection matmuls, Wc DMA, and their PSUM traffic disappear.
  - bk dropped on device: softmax over k is invariant to the per-q additive
    shift (bk . q), so the keys bias cancels exactly.
  - bv folded into bc on host: sum_k attn = 1 implies ctx = ctx0 + bv, so
    out = ctx0 @ Wc + (bc + bv.flatten() @ Wc). Removes all vals-bias work.
  - scores/keys/vals/transposes share a 4-bank PSUM pool (phases disjoint);
    denominators 2 banks; ctx accumulators 2 banks. 8 banks total.
  - per head, the kt loop emits scores+exp for kt and ctx matmuls for kt-1
    (software pipelining) so PE never waits on ACT's exp.
  - head h's normalize (recip + mul + accumulate into final) is deferred
    until after head h+1's keys so DVE latency hides under PE matmuls.
"""

import sys

for _p in ("/opt/trn_rl_repo",):
    if _p not in sys.path:
        sys.path.insert(0, _p)

import numpy as np

import concourse.bass as bass
import concourse.mybir as mybir
import concourse.tile as tile
from concourse import bacc, bass_utils
from concourse.masks import make_identity

B, SQ, SK = 8, 1024, 1024
D, E, H = 1024, 128, 32
P = 128          # partition width
DCH = D // P     # 8 d-chunks
KT = SK // P     # 8 k-tiles
G = 4            # heads per vals-group
NG = H // G      # 8 groups
NHALF = 512      # matmul moving-dim chunk (one PSUM bank of fp32)
SCALE = 1.0 / float(np.sqrt(E))

F32 = mybir.dt.float32
F16 = mybir.dt.float16

N_CORES = 8

_COMPILED = {}
_ONES_SQ = np.ones((P, P), np.float16)


def build_nc(mm_dtype="f16", repeat=1):
    """Build the single-core Bass program (SPMD across 8 cores).

    repeat > 1 re-emits the whole computation that many times (identical
    work each pass) for launch-overhead-amortized timing; the final DRAM
    output is written by every pass (all identical).
    """
    MT = F16

    nc = bacc.Bacc("TRN2", target_bir_lowering=False, debug=False)

    statesT = nc.dram_tensor("statesT", [D, SK], MT, kind="ExternalInput").ap()
    queryT = nc.dram_tensor("queryT", [E, SQ], MT, kind="ExternalInput").ap()
    WkT = nc.dram_tensor("WkT", [NG, D, G * E], MT, kind="ExternalInput").ap()
    WvcT = nc.dram_tensor("WvcT", [NG, D, G * E], MT, kind="ExternalInput").ap()
    bcT = nc.dram_tensor("bcT", [E, 1], F32, kind="ExternalInput").ap()
    onesSQ = nc.dram_tensor("onesSQ", [P, P], MT, kind="ExternalInput").ap()
    out = nc.dram_tensor("out", [SQ, E], F32, kind="ExternalOutput").ap()

    from contextlib import ExitStack

    with tile.TileContext(nc) as tc, ExitStack() as es:
        constp = es.enter_context(tc.tile_pool(name="const", bufs=1))
        statesp = es.enter_context(tc.tile_pool(name="states", bufs=DCH))
        queryp = es.enter_context(tc.tile_pool(name="query", bufs=1))
        wkp = es.enter_context(tc.tile_pool(name="wk", bufs=10))
        wvp = es.enter_context(tc.tile_pool(name="wv", bufs=10))
        keysp = es.enter_context(tc.tile_pool(name="keys", bufs=2))
        expp = es.enter_context(tc.tile_pool(name="exps", bufs=8))
        valsp = es.enter_context(tc.tile_pool(name="vals", bufs=12))
        recipp = es.enter_context(tc.tile_pool(name="recip", bufs=2))
        tmpp = es.enter_context(tc.tile_pool(name="tmpn", bufs=2))
        finalp = es.enter_context(tc.tile_pool(name="final", bufs=1))
        outp = es.enter_context(tc.tile_pool(name="outs", bufs=4))
        dsump = es.enter_context(tc.tile_pool(name="dsum", bufs=12))
        ps_main = es.enter_context(tc.tile_pool(name="ps_main", bufs=4, space="PSUM"))
        ps_denom = es.enter_context(tc.tile_pool(name="ps_denom", bufs=2, space="PSUM"))
        ps_ctx = es.enter_context(tc.tile_pool(name="ps_ctx", bufs=2, space="PSUM"))

        # ---- constants ----
        ones_sq = constp.tile([P, P], MT)
        nc.sync.dma_start(ones_sq[:], onesSQ[:])
        ident = constp.tile([P, P], F32)
        make_identity(nc, ident[:])
        bc_t = constp.tile([E, 1], F32)
        nc.sync.dma_start(bc_t[:], bcT[:])

        # ---- resident activations ----
        st = []
        for d in range(DCH):
            st_t = statesp.tile([P, SK], MT, name="st_t")
            nc.sync.dma_start(st_t[:], statesT[d * P : (d + 1) * P, :])
            st.append(st_t)
        q_t = queryp.tile([E, SQ], MT)
        nc.sync.dma_start(q_t[:], queryT[:])

        final_t = finalp.tile([E, SQ], F32)

        def emit_norm(h, pd, pc):
            """Normalize head h's context and accumulate into final_t."""
            for qh in range(2):
                rec = recipp.tile([P, NHALF], F32, name="rec")
                nc.vector.reciprocal_approx_fast(out=rec[:], in_=pd[qh][:])
                tmp = tmpp.tile([P, NHALF], F32, name="tmp")
                nc.vector.tensor_mul(tmp[:], pc[qh][:], rec[:])
                if h == 0:
                    nc.vector.tensor_scalar(
                        final_t[:, qh * NHALF : (qh + 1) * NHALF],
                        tmp[:],
                        bc_t[:],
                        None,
                        op0=mybir.AluOpType.add,
                    )
                else:
                    nc.vector.tensor_add(
                        final_t[:, qh * NHALF : (qh + 1) * NHALF],
                        final_t[:, qh * NHALF : (qh + 1) * NHALF],
                        tmp[:],
                    )

        pending_norm = None

        for rep in range(repeat):
            def emit_wdma(gw):
                wvt, wkt = [], []
                for d in range(DCH):
                    wv_t = wvp.tile([P, G * E], MT, name="wv_t")
                    nc.sync.dma_start(wv_t[:], WvcT[gw, d * P : (d + 1) * P, :])
                    wvt.append(wv_t)
                for d in range(DCH):
                    wk_t = wkp.tile([P, G * E], MT, name="wk_t")
                    nc.sync.dma_start(wk_t[:], WkT[gw, d * P : (d + 1) * P, :])
                    wkt.append(wk_t)
                return wvt, wkt

            def emit_vals_tile(kt, wvt):
                pv = ps_main.tile([P, G * E], F32, tag="m", name="pv")
                for d in range(DCH):
                    nc.tensor.matmul(
                        pv[:],
                        (st[d][:, kt * P : (kt + 1) * P]),
                        (wvt[d][:]),
                        start=(d == 0),
                        stop=(d == DCH - 1),
                    )
                v_sb = valsp.tile([P, G * E], MT, name="v_sb")
                nc.vector.tensor_copy(v_sb[:], pv[:])
                return v_sb

            next_w = emit_wdma(0)
            next_vals = []

            for g in range(NG):
                wv_tiles, wk_tiles = next_w
                # vals tiles not already built during the previous group's
                # last head:
                vals_tiles = next_vals
                for kt in range(len(vals_tiles), KT):
                    vals_tiles.append(emit_vals_tile(kt, wv_tiles))
                next_vals = []

                def emit_keys_block(hg_k):
                    """Full 16-matmul keys emission for head (g, hg_k)."""
                    ksb = keysp.tile([E, SK], MT, name="keys_sb")
                    for half in range(2):
                        pk = ps_main.tile(
                            [P, NHALF], F32, tag="m", name="pk"
                        )
                        for d in range(DCH):
                            nc.tensor.matmul(
                                pk[:],
                                (wk_tiles[d][:, hg_k * E : (hg_k + 1) * E]),
                                (st[d][:, half * NHALF : (half + 1) * NHALF]),
                                start=(d == 0),
                                stop=(d == DCH - 1),
                            )
                        nc.vector.tensor_copy(
                            ksb[:, half * NHALF : (half + 1) * NHALF], pk[:]
                        )
                    return ksb

                # first head's keys as one block (nothing to hide them under)
                next_keys_sb = emit_keys_block(0)

                for hg in range(G):
                    h = g * G + hg
                    keys_sb = next_keys_sb

                    # previous head's normalize: scheduled into the early
                    # kt-phase so DVE recip/mul latency hides under PE.
                    if pending_norm is not None:
                        emit_norm(*pending_norm)
                        pending_norm = None

                    # next head's keys are interleaved into this head's kt
                    # loop (2 matmuls per kt) to keep PE saturated while the
                    # softmax phase is ACT-paced. kts 0-3 build half 0,
                    # kts 4-7 half 1.
                    ik_hg = hg + 1 if hg + 1 < G else None
                    if ik_hg is not None:
                        next_keys_sb = keysp.tile([E, SK], MT, name="keys_sb")
                        ik_pk = None
                    iv_w = None
                    if ik_hg is None and g + 1 < NG:
                        next_w = emit_wdma(g + 1)
                        iv_w = next_w[0]
                        iv_pv = None

                    # ---- scores -> exp -> denom/ctx accumulate, per (kt, qh) ----
                    pd = [
                        ps_denom.tile(
                            [P, NHALF], F32, tag="denom", name="pd"
                        )
                        for i in range(2)
                    ]
                    pc = [
                        ps_ctx.tile(
                            [P, NHALF], F32, tag="ctx", name="pc"
                        )
                        for i in range(2)
                    ]
                    # Software-pipelined: emit kt's scores+exp, then kt-1's
                    # ctx matmuls (so PE never waits on the just-issued exp).
                    # Denominator: gpsimd pair-adds of exp tiles as pairs
                    # complete (Pool engine is otherwise idle), DVE finishes
                    # the reduction tree, then ONE ones-matmul per half.
                    pending = None  # ([ex_qh0, ex_qh1], kt)
                    last_ex = [None, None]
                    partials = [[], []]
                    for kt in range(KT):
                        exs = []
                        for qh in range(2):
                            ps = ps_main.tile(
                                [P, NHALF], F32, tag="m",
                                name="ps",
                            )
                            nc.tensor.matmul(
                                ps[:],
                                (keys_sb[:, kt * P : (kt + 1) * P]),
                                (q_t[:, qh * NHALF : (qh + 1) * NHALF]),
                                start=True,
                                stop=True,
                            )
                            ex = expp.tile(
                                [P, NHALF], MT, name="ex"
                            )
                            nc.scalar.activation(
                                ex[:], ps[:], mybir.ActivationFunctionType.Exp,
                                scale=SCALE,
                            )
                            exs.append(ex)
                            if kt % 2 == 1:
                                gsum = dsump.tile([P, NHALF], MT, name="gsum")
                                nc.gpsimd.tensor_add(
                                    gsum[:], last_ex[qh][:], ex[:]
                                )
                                partials[qh].append(gsum)
                            else:
                                last_ex[qh] = ex
                        if iv_w is not None:
                            # 2 vals matmuls of group g+1 per kt: tile kt//4,
                            # d-chunks 2*(kt%4) and 2*(kt%4)+1
                            vkt = kt // 4
                            loc = kt % 4
                            if loc == 0:
                                iv_pv = ps_main.tile(
                                    [P, G * E], F32, tag="m", name="pv"
                                )
                            for d in (2 * loc, 2 * loc + 1):
                                nc.tensor.matmul(
                                    iv_pv[:],
                                    (st[d][:, vkt * P : (vkt + 1) * P]),
                                    (iv_w[d][:]),
                                    start=(d == 0),
                                    stop=(d == DCH - 1),
                                )
                            if loc == 3:
                                v_sb = valsp.tile([P, G * E], MT, name="v_sb")
                                nc.vector.tensor_copy(v_sb[:], iv_pv[:])
                                next_vals.append(v_sb)
                        if ik_hg is not None:
                            half = kt // 4
                            loc = kt % 4
                            if loc == 0:
                                ik_pk = ps_main.tile(
                                    [P, NHALF], F32, tag="m", name="pk"
                                )
                            for d in (2 * loc, 2 * loc + 1):
                                nc.tensor.matmul(
                                    ik_pk[:],
                                    (wk_tiles[d][:, ik_hg * E : (ik_hg + 1) * E]),
                                    (st[d][:, half * NHALF : (half + 1) * NHALF]),
                                    start=(d == 0),
                                    stop=(d == DCH - 1),
                                )
                            if loc == 3:
                                nc.vector.tensor_copy(
                                    next_keys_sb[
                                        :, half * NHALF : (half + 1) * NHALF
                                    ],
                                    ik_pk[:],
                                )
                        if pending is not None:
                            pexs, pkt = pending
                            for qh in range(2):
                                nc.tensor.matmul(
                                    pc[qh][:],
                                    (vals_tiles[pkt][:, hg * E : (hg + 1) * E]),
                                    (pexs[qh][:]),
                                    start=(pkt == 0),
                                    stop=False,
                                )
                        pending = (exs, kt)
                    pexs, pkt = pending
                    for qh in range(2):
                        nc.tensor.matmul(
                            pc[qh][:],
                            (vals_tiles[pkt][:, hg * E : (hg + 1) * E]),
                            (pexs[qh][:]),
                            start=False,
                            stop=True,
                        )
                    for qh in range(2):
                        p0, p1, p2, p3 = partials[qh]
                        d1 = dsump.tile([P, NHALF], MT, name="gsum")
                        nc.vector.tensor_add(d1[:], p0[:], p1[:])
                        d2 = dsump.tile([P, NHALF], MT, name="gsum")
                        nc.vector.tensor_add(d2[:], p2[:], p3[:])
                        ssum = dsump.tile([P, NHALF], MT, name="gsum")
                        nc.vector.tensor_add(ssum[:], d1[:], d2[:])
                        nc.tensor.matmul(
                            pd[qh][:], (ones_sq[:]), (ssum[:]),
                            start=True, stop=True,
                        )

                    # defer normalize+accumulate until after next head's keys
                    pending_norm = (h, pd, pc)

            emit_norm(*pending_norm)
            pending_norm = None

            # ---- transpose finalT -> out [SQ, E] ----
            for qt in range(KT):
                pt = ps_main.tile([P, P], F32, tag="m", name="pt")
                nc.tensor.transpose(
                    pt[:], final_t[:, qt * P : (qt + 1) * P], ident[:]
                )
                o_sb = outp.tile([P, E], F32, name="o_sb")
                nc.vector.tensor_copy(o_sb[:], pt[:])
                nc.sync.dma_start(out[qt * P : (qt + 1) * P, :], o_sb[:])

    nc.compile()
    return nc


def _prep_inputs(query, states, Wk, bk, Wv, bv, Wc, bc):
    """Host-side sharding: per-core input maps (core c == batch element c).

    bk is dropped (softmax shift invariance); bv is folded into bc; Wc is
    folded into Wv (Wvc_h = Wv_h @ Wc_h):
    out = sum_h attn_h @ (states @ Wvc_h) + (bc + bv.flatten() @ Wc).
    """
    query = np.asarray(query, np.float32)
    states = np.asarray(states, np.float32)
    Wk = np.asarray(Wk, np.float64)
    Wv = np.asarray(Wv, np.float64)
    Wc = np.asarray(Wc, np.float64)
    bv = np.asarray(bv, np.float64)
    bc = np.asarray(bc, np.float64)

    # Wvc[h] = Wv[h] @ Wc[h*E:(h+1)*E, :]  -> [H, D, E]
    Wvc = np.einsum("hde,hef->hdf", Wv, Wc.reshape(H, E, E))

    WkT = np.ascontiguousarray(
        Wk.transpose(1, 0, 2).reshape(D, NG, G * E).transpose(1, 0, 2)
    ).astype(np.float16)
    WvcT = np.ascontiguousarray(
        Wvc.transpose(1, 0, 2).reshape(D, NG, G * E).transpose(1, 0, 2)
    ).astype(np.float16)
    bc_eff = (bc + bv.reshape(H * E) @ Wc).astype(np.float32)
    bcT = np.ascontiguousarray(bc_eff.reshape(E, 1))

    in_maps = []
    for c in range(N_CORES):
        in_maps.append(
            {
                "statesT": np.ascontiguousarray(states[c].T).astype(np.float16),
                "queryT": np.ascontiguousarray(query[c].T).astype(np.float16),
                "WkT": WkT,
                "WvcT": WvcT,
                "bcT": bcT,
                "onesSQ": _ONES_SQ,
            }
        )
    return in_maps


def get_nc(mm_dtype="f16", repeat=1):
    key = (mm_dtype, repeat)
    nc = _COMPILED.get(key)
    if nc is None:
        nc = build_nc(mm_dtype, repeat=repeat)
        _COMPILED[key] = nc
    return nc


def kernel(query, states, Wk, bk, Wv, bv, Wc, bc):
    nc = get_nc()
    in_maps = _prep_inputs(query, states, Wk, bk, Wv, bv, Wc, bc)
    res = bass_utils.run_bass_kernel_spmd(nc, in_maps, list(range(N_CORES)))
    return np.stack([res.results[c]["out"] for c in range(N_CORES)], axis=0)


# revision 7
# speedup vs baseline: 1.2249x; 1.1260x over previous
"""Multi-head attention (lazy K/V projections) Trainium2 Bass kernel.

Problem: nn_MultiHeadAttention_54520314856024
  B=8, SQ=SK=1024, D=1024, E=128, H=32
  keys  = einsum('bsd,hde->hbse', states, Wk) + bk
  vals  = einsum('bsd,hde->hbse', states, Wv) + bv
  attn  = softmax(einsum('bqe,hbke->hbqk', query, keys) / sqrt(E))
  ctx   = einsum('hbqk,hbke->hbqe', attn, vals) -> concat heads -> @ Wc + bc
  This is out = sum_h softmax(q keys_h^T) (states Wv_h Wc_h) + bc_eff.

Sharding: batch-parallel, one batch element per NeuronCore (8 cores).

Design notes:
  - fp16 matmul inputs: same 1 cycle/row PE rate as f32r but enables FWL
    (fast weight load) so LDWEIGHTS hides under matmuls, and 2x DVE/GpSimd
    elementwise rates + half the DMA bytes. fp16 (10-bit mantissa) over
    bf16 for accuracy margin; all tensors here are O(1) so no range risk.
  - Wc folded into Wv on host: vc_h = states @ (Wv_h Wc_h), so the per-head
    normalized context IS the head's output contribution; the final [H*E,E]
    projection matmuls, Wc DMA, and their PSUM traffic disappear.
  - bk dropped on device: softmax over k is invariant to the per-q additive
    shift (bk . q), so the keys bias cancels exactly.
  - bv folded into bc on host: sum_k attn = 1 implies ctx = ctx0 + bv, so
    out = ctx0 @ Wc + (bc + bv.flatten() @ Wc). Removes all vals-bias work.
  - scores/keys/vals/transposes share a 4-bank PSUM pool (phases disjoint);
    denominators 2 banks; ctx accumulators 2 banks. 8 banks total.
  - per head, the kt loop emits scores+exp for kt and ctx matmuls for kt-1
    (software pipelining) so PE never waits on ACT's exp.
  - head h's normalize (recip + mul + accumulate into final) is deferred
    until after head h+1's keys so DVE latency hides under PE matmuls.
"""

import sys

for _p in ("/opt/trn_rl_repo",):
    if _p not in sys.path:
        sys.path.insert(0, _p)

import numpy as np

import concourse.bass as bass
import concourse.mybir as mybir
import concourse.tile as tile
from concourse import bacc, bass_utils
from concourse.masks import make_identity

B, SQ, SK = 8, 1024, 1024
D, E, H = 1024, 128, 32
P = 128          # partition width
DCH = D // P     # 8 d-chunks
KT = SK // P     # 8 k-tiles
G = 4            # heads per vals-group
NG = H // G      # 8 groups
NHALF = 512      # matmul moving-dim chunk (one PSUM bank of fp32)
SCALE = 1.0 / float(np.sqrt(E))

F32 = mybir.dt.float32
BF16 = mybir.dt.bfloat16

N_CORES = 8

_COMPILED = {}
import ml_dtypes
_ONES_SQ = np.ones((P, P), ml_dtypes.bfloat16)


def build_nc(mm_dtype="bf16", repeat=1):
    """Build the single-core Bass program (SPMD across 8 cores).

    repeat > 1 re-emits the whole computation that many times (identical
    work each pass) for launch-overhead-amortized timing; the final DRAM
    output is written by every pass (all identical).
    """
    MT = BF16

    nc = bacc.Bacc("TRN2", target_bir_lowering=False, debug=False)

    statesT = nc.dram_tensor("statesT", [D, SK], MT, kind="ExternalInput").ap()
    queryT = nc.dram_tensor("queryT", [E, SQ], MT, kind="ExternalInput").ap()
    WkT = nc.dram_tensor("WkT", [NG, D, G * E], MT, kind="ExternalInput").ap()
    WvcT = nc.dram_tensor("WvcT", [NG, D, G * E], MT, kind="ExternalInput").ap()
    bcT = nc.dram_tensor("bcT", [E, 1], F32, kind="ExternalInput").ap()
    onesSQ = nc.dram_tensor("onesSQ", [P, P], MT, kind="ExternalInput").ap()
    out = nc.dram_tensor("out", [SQ, E], F32, kind="ExternalOutput").ap()

    from contextlib import ExitStack

    with tile.TileContext(nc) as tc, ExitStack() as es:
        constp = es.enter_context(tc.tile_pool(name="const", bufs=1))
        statesp = es.enter_context(tc.tile_pool(name="states", bufs=DCH))
        queryp = es.enter_context(tc.tile_pool(name="query", bufs=1))
        wkp = es.enter_context(tc.tile_pool(name="wk", bufs=10))
        wvp = es.enter_context(tc.tile_pool(name="wv", bufs=10))
        keysp = es.enter_context(tc.tile_pool(name="keys", bufs=2))
        expp = es.enter_context(tc.tile_pool(name="exps", bufs=8))
        valsp = es.enter_context(tc.tile_pool(name="vals", bufs=12))
        recipp = es.enter_context(tc.tile_pool(name="recip", bufs=2))
        tmpp = es.enter_context(tc.tile_pool(name="tmpn", bufs=2))
        finalp = es.enter_context(tc.tile_pool(name="final", bufs=1))
        outp = es.enter_context(tc.tile_pool(name="outs", bufs=4))
        dsump = es.enter_context(tc.tile_pool(name="dsum", bufs=12))
        ps_main = es.enter_context(tc.tile_pool(name="ps_main", bufs=4, space="PSUM"))
        ps_denom = es.enter_context(tc.tile_pool(name="ps_denom", bufs=2, space="PSUM"))
        ps_ctx = es.enter_context(tc.tile_pool(name="ps_ctx", bufs=2, space="PSUM"))

        # ---- constants ----
        ones_sq = constp.tile([P, P], MT)
        nc.sync.dma_start(ones_sq[:], onesSQ[:])
        ident = constp.tile([P, P], F32)
        make_identity(nc, ident[:])
        bc_t = constp.tile([E, 1], F32)
        nc.sync.dma_start(bc_t[:], bcT[:])

        # ---- resident activations ----
        st = []
        for d in range(DCH):
            st_t = statesp.tile([P, SK], MT, name="st_t")
            nc.sync.dma_start(st_t[:], statesT[d * P : (d + 1) * P, :])
            st.append(st_t)
        q_t = queryp.tile([E, SQ], MT)
        nc.sync.dma_start(q_t[:], queryT[:])

        final_t = finalp.tile([E, SQ], F32)

        def emit_norm(h, pd, pc, ssums):
            """Denominator matmuls + normalize head h's context into final_t.

            Deferred into the NEXT head's kt loop so the PE queue never
            stalls on the (GpSimd+DVE) exp-reduction tree: by the time these
            ones-matmuls are issued, ssum has long been ready.
            """
            for qh in range(2):
                nc.tensor.matmul(
                    pd[qh][:], (ones_sq[:]), (ssums[qh][:]),
                    start=True, stop=True,
                )
            for qh in range(2):
                rec = recipp.tile([P, NHALF], F32, name="rec")
                nc.vector.reciprocal_approx_fast(out=rec[:], in_=pd[qh][:])
                tmp = tmpp.tile([P, NHALF], F32, name="tmp")
                nc.vector.tensor_mul(tmp[:], pc[qh][:], rec[:])
                if h == 0:
                    nc.vector.tensor_scalar(
                        final_t[:, qh * NHALF : (qh + 1) * NHALF],
                        tmp[:],
                        bc_t[:],
                        None,
                        op0=mybir.AluOpType.add,
                    )
                else:
                    nc.vector.tensor_add(
                        final_t[:, qh * NHALF : (qh + 1) * NHALF],
                        final_t[:, qh * NHALF : (qh + 1) * NHALF],
                        tmp[:],
                    )

        pending_norm = None

        for rep in range(repeat):
            def emit_wdma(gw):
                wvt, wkt = [], []
                for d in range(DCH):
                    wv_t = wvp.tile([P, G * E], MT, name="wv_t")
                    nc.sync.dma_start(wv_t[:], WvcT[gw, d * P : (d + 1) * P, :])
                    wvt.append(wv_t)
                for d in range(DCH):
                    wk_t = wkp.tile([P, G * E], MT, name="wk_t")
                    nc.sync.dma_start(wk_t[:], WkT[gw, d * P : (d + 1) * P, :])
                    wkt.append(wk_t)
                return wvt, wkt

            def emit_vals_tile(kt, wvt):
                pv = ps_main.tile([P, G * E], F32, tag="m", name="pv")
                for d in range(DCH):
                    nc.tensor.matmul(
                        pv[:],
                        (st[d][:, kt * P : (kt + 1) * P]),
                        (wvt[d][:]),
                        start=(d == 0),
                        stop=(d == DCH - 1),
                    )
                v_sb = valsp.tile([P, G * E], MT, name="v_sb")
                nc.vector.tensor_copy(v_sb[:], pv[:])
                return v_sb

            next_w = emit_wdma(0)
            next_vals = []

            for g in range(NG):
                wv_tiles, wk_tiles = next_w
                # vals tiles not already built during the previous group's
                # last head:
                vals_tiles = next_vals
                for kt in range(len(vals_tiles), KT):
                    vals_tiles.append(emit_vals_tile(kt, wv_tiles))
                next_vals = []

                def emit_keys_block(hg_k):
                    """Full 16-matmul keys emission for head (g, hg_k)."""
                    ksb = keysp.tile([E, SK], MT, name="keys_sb")
                    for half in range(2):
                        pk = ps_main.tile(
                            [P, NHALF], F32, tag="m", name="pk"
                        )
                        for d in range(DCH):
                            nc.tensor.matmul(
                                pk[:],
                                (wk_tiles[d][:, hg_k * E : (hg_k + 1) * E]),
                                (st[d][:, half * NHALF : (half + 1) * NHALF]),
                                start=(d == 0),
                                stop=(d == DCH - 1),
                            )
                        nc.vector.tensor_copy(
                            ksb[:, half * NHALF : (half + 1) * NHALF], pk[:]
                        )
                    return ksb

                # first head's keys as one block (nothing to hide them under)
                next_keys_sb = emit_keys_block(0)

                for hg in range(G):
                    h = g * G + hg
                    keys_sb = next_keys_sb

                    # next head's keys are interleaved into this head's kt
                    # loop (2 matmuls per kt) to keep PE saturated while the
                    # softmax phase is ACT-paced. kts 0-3 build half 0,
                    # kts 4-7 half 1.
                    ik_hg = hg + 1 if hg + 1 < G else None
                    if ik_hg is not None:
                        next_keys_sb = keysp.tile([E, SK], MT, name="keys_sb")
                        ik_pk = None
                    iv_w = None
                    if ik_hg is None and g + 1 < NG:
                        next_w = emit_wdma(g + 1)
                        iv_w = next_w[0]
                        iv_pv = None

                    # ---- scores -> exp -> denom/ctx accumulate, per (kt, qh) ----
                    pd = [
                        ps_denom.tile(
                            [P, NHALF], F32, tag="denom", name="pd"
                        )
                        for i in range(2)
                    ]
                    pc = [
                        ps_ctx.tile(
                            [P, NHALF], F32, tag="ctx", name="pc"
                        )
                        for i in range(2)
                    ]
                    # Software-pipelined: emit kt's scores+exp, then kt-1's
                    # ctx matmuls (so PE never waits on the just-issued exp).
                    # Denominator: gpsimd pair-adds of exp tiles as pairs
                    # complete (Pool engine is otherwise idle), DVE finishes
                    # the reduction tree, then ONE ones-matmul per half.
                    pending = None  # ([ex_qh0, ex_qh1], kt)
                    last_ex = [None, None]
                    partials = [[], []]
                    for kt in range(KT):
                        if kt == 1 and pending_norm is not None:
                            # previous head's denominator matmuls + normalize,
                            # deferred here so PE never stalls on its tree.
                            emit_norm(*pending_norm)
                            pending_norm = None
                        exs = []
                        for qh in range(2):
                            ps = ps_main.tile(
                                [P, NHALF], F32, tag="m",
                                name="ps",
                            )
                            nc.tensor.matmul(
                                ps[:],
                                (keys_sb[:, kt * P : (kt + 1) * P]),
                                (q_t[:, qh * NHALF : (qh + 1) * NHALF]),
                                start=True,
                                stop=True,
                            )
                            ex = expp.tile(
                                [P, NHALF], MT, name="ex"
                            )
                            nc.scalar.activation(
                                ex[:], ps[:], mybir.ActivationFunctionType.Exp,
                                scale=SCALE,
                            )
                            exs.append(ex)
                            if kt % 2 == 1:
                                gsum = dsump.tile([P, NHALF], MT, name="gsum")
                                nc.gpsimd.tensor_add(
                                    gsum[:], last_ex[qh][:], ex[:]
                                )
                                partials[qh].append(gsum)
                            else:
                                last_ex[qh] = ex
                        if iv_w is not None:
                            # 2 vals matmuls of group g+1 per kt: tile kt//4,
                            # d-chunks 2*(kt%4) and 2*(kt%4)+1
                            vkt = kt // 4
                            loc = kt % 4
                            if loc == 0:
                                iv_pv = ps_main.tile(
                                    [P, G * E], F32, tag="m", name="pv"
                                )
                            for d in (2 * loc, 2 * loc + 1):
                                nc.tensor.matmul(
                                    iv_pv[:],
                                    (st[d][:, vkt * P : (vkt + 1) * P]),
                                    (iv_w[d][:]),
                                    start=(d == 0),
                                    stop=(d == DCH - 1),
                                )
                            if loc == 3:
                                v_sb = valsp.tile([P, G * E], MT, name="v_sb")
                                nc.vector.tensor_copy(v_sb[:], iv_pv[:])
                                next_vals.append(v_sb)
                        if ik_hg is not None:
                            half = kt // 4
                            loc = kt % 4
                            if loc == 0:
                                ik_pk = ps_main.tile(
                                    [P, NHALF], F32, tag="m", name="pk"
                                )
                            for d in (2 * loc, 2 * loc + 1):
                                nc.tensor.matmul(
                                    ik_pk[:],
                                    (wk_tiles[d][:, ik_hg * E : (ik_hg + 1) * E]),
                                    (st[d][:, half * NHALF : (half + 1) * NHALF]),
                                    start=(d == 0),
                                    stop=(d == DCH - 1),
                                )
                            if loc == 3:
                                nc.vector.tensor_copy(
                                    next_keys_sb[
                                        :, half * NHALF : (half + 1) * NHALF
                                    ],
                                    ik_pk[:],
                                )
                        if pending is not None:
                            pexs, pkt = pending
                            for qh in range(2):
                                nc.tensor.matmul(
                                    pc[qh][:],
                                    (vals_tiles[pkt][:, hg * E : (hg + 1) * E]),
                                    (pexs[qh][:]),
                                    start=(pkt == 0),
                                    stop=False,
                                )
                        pending = (exs, kt)
                    pexs, pkt = pending
                    for qh in range(2):
                        nc.tensor.matmul(
                            pc[qh][:],
                            (vals_tiles[pkt][:, hg * E : (hg + 1) * E]),
                            (pexs[qh][:]),
                            start=False,
                            stop=True,
                        )
                    ssums = []
                    for qh in range(2):
                        p0, p1, p2, p3 = partials[qh]
                        d1 = dsump.tile([P, NHALF], MT, name="gsum")
                        nc.vector.tensor_add(d1[:], p0[:], p1[:])
                        d2 = dsump.tile([P, NHALF], MT, name="gsum")
                        nc.vector.tensor_add(d2[:], p2[:], p3[:])
                        ssum = dsump.tile([P, NHALF], MT, name="gsum")
                        nc.vector.tensor_add(ssum[:], d1[:], d2[:])
                        ssums.append(ssum)

                    # defer denom matmuls + normalize into next head's kt loop
                    pending_norm = (h, pd, pc, ssums)

            emit_norm(*pending_norm)
            pending_norm = None

            # ---- transpose finalT -> out [SQ, E] ----
            for qt in range(KT):
                pt = ps_main.tile([P, P], F32, tag="m", name="pt")
                nc.tensor.transpose(
                    pt[:], final_t[:, qt * P : (qt + 1) * P], ident[:]
                )
                o_sb = outp.tile([P, E], F32, name="o_sb")
                nc.vector.tensor_copy(o_sb[:], pt[:])
                nc.sync.dma_start(out[qt * P : (qt + 1) * P, :], o_sb[:])

    nc.compile()
    return nc


def _prep_inputs(query, states, Wk, bk, Wv, bv, Wc, bc):
    """Host-side sharding: per-core input maps (core c == batch element c).

    bk is dropped (softmax shift invariance); bv is folded into bc; Wc is
    folded into Wv (Wvc_h = Wv_h @ Wc_h):
    out = sum_h attn_h @ (states @ Wvc_h) + (bc + bv.flatten() @ Wc).
    """
    query = np.asarray(query, np.float32)
    states = np.asarray(states, np.float32)
    Wk = np.asarray(Wk, np.float64)
    Wv = np.asarray(Wv, np.float64)
    Wc = np.asarray(Wc, np.float64)
    bv = np.asarray(bv, np.float64)
    bc = np.asarray(bc, np.float64)

    # Wvc[h] = Wv[h] @ Wc[h*E:(h+1)*E, :]  -> [H, D, E]
    Wvc = np.einsum("hde,hef->hdf", Wv, Wc.reshape(H, E, E))

    WkT = np.ascontiguousarray(
        Wk.transpose(1, 0, 2).reshape(D, NG, G * E).transpose(1, 0, 2)
    ).astype(ml_dtypes.bfloat16)
    WvcT = np.ascontiguousarray(
        Wvc.transpose(1, 0, 2).reshape(D, NG, G * E).transpose(1, 0, 2)
    ).astype(ml_dtypes.bfloat16)
    bc_eff = (bc + bv.reshape(H * E) @ Wc).astype(np.float32)
    bcT = np.ascontiguousarray(bc_eff.reshape(E, 1))

    in_maps = []
    for c in range(N_CORES):
        in_maps.append(
            {
                "statesT": np.ascontiguousarray(states[c].T).astype(ml_dtypes.bfloat16),
                "queryT": np.ascontiguousarray(query[c].T).astype(ml_dtypes.bfloat16),
                "WkT": WkT,
                "WvcT": WvcT,
                "bcT": bcT,
                "onesSQ": _ONES_SQ,
            }
        )
    return in_maps


def get_nc(mm_dtype="bf16", repeat=1):
    key = (mm_dtype, repeat)
    nc = _COMPILED.get(key)
    if nc is None:
        nc = build_nc(mm_dtype, repeat=repeat)
        _COMPILED[key] = nc
    return nc


def kernel(query, states, Wk, bk, Wv, bv, Wc, bc):
    nc = get_nc()
    in_maps = _prep_inputs(query, states, Wk, bk, Wv, bv, Wc, bc)
    res = bass_utils.run_bass_kernel_spmd(nc, in_maps, list(range(N_CORES)))
    return np.stack([res.results[c]["out"] for c in range(N_CORES)], axis=0)


# revision 13
# speedup vs baseline: 1.2890x; 1.0524x over previous
"""Multi-head attention (lazy K/V projections) Trainium2 Bass kernel.

Problem: nn_MultiHeadAttention_54520314856024
  B=8, SQ=SK=1024, D=1024, E=128, H=32
  keys  = einsum('bsd,hde->hbse', states, Wk) + bk
  vals  = einsum('bsd,hde->hbse', states, Wv) + bv
  attn  = softmax(einsum('bqe,hbke->hbqk', query, keys) / sqrt(E))
  ctx   = einsum('hbqk,hbke->hbqe', attn, vals) -> concat heads -> @ Wc + bc
  This is out = sum_h softmax(q keys_h^T) (states Wv_h Wc_h) + bc_eff.

Sharding: batch-parallel, one batch element per NeuronCore (8 cores).

Design notes:
  - fp16 matmul inputs: same 1 cycle/row PE rate as f32r but enables FWL
    (fast weight load) so LDWEIGHTS hides under matmuls, and 2x DVE/GpSimd
    elementwise rates + half the DMA bytes. fp16 (10-bit mantissa) over
    bf16 for accuracy margin; all tensors here are O(1) so no range risk.
  - Wc folded into Wv on host: vc_h = states @ (Wv_h Wc_h), so the per-head
    normalized context IS the head's output contribution; the final [H*E,E]
    projection matmuls, Wc DMA, and their PSUM traffic disappear.
  - bk dropped on device: softmax over k is invariant to the per-q additive
    shift (bk . q), so the keys bias cancels exactly.
  - bv folded into bc on host: sum_k attn = 1 implies ctx = ctx0 + bv, so
    out = ctx0 @ Wc + (bc + bv.flatten() @ Wc). Removes all vals-bias work.
  - scores/keys/vals/transposes share a 4-bank PSUM pool (phases disjoint);
    denominators 2 banks; ctx accumulators 2 banks. 8 banks total.
  - per head, the kt loop emits scores+exp for kt and ctx matmuls for kt-1
    (software pipelining) so PE never waits on ACT's exp.
  - head h's normalize (recip + mul + accumulate into final) is deferred
    until after head h+1's keys so DVE latency hides under PE matmuls.
"""

import sys

for _p in ("/opt/trn_rl_repo",):
    if _p not in sys.path:
        sys.path.insert(0, _p)

import numpy as np

import concourse.bass as bass
import concourse.mybir as mybir
import concourse.tile as tile
from concourse import bacc, bass_utils
from concourse.masks import make_identity

B, SQ, SK = 8, 1024, 1024
D, E, H = 1024, 128, 32
P = 128          # partition width
DCH = D // P     # 8 d-chunks
KT = SK // P     # 8 k-tiles
G = 4            # heads per vals-group
NG = H // G      # 8 groups
NHALF = 512      # matmul moving-dim chunk (one PSUM bank of fp32)
SCALE = 1.0 / float(np.sqrt(E))

F32 = mybir.dt.float32
BF16 = mybir.dt.bfloat16

N_CORES = 8

_COMPILED = {}
import ml_dtypes
_ONES_SQ = np.ones((P, P), ml_dtypes.bfloat16)


def build_nc(mm_dtype="bf16", repeat=1):
    """Build the single-core Bass program (SPMD across 8 cores).

    repeat > 1 re-emits the whole computation that many times (identical
    work each pass) for launch-overhead-amortized timing; the final DRAM
    output is written by every pass (all identical).
    """
    MT = BF16

    nc = bacc.Bacc("TRN2", target_bir_lowering=False, debug=False)

    statesT = nc.dram_tensor("statesT", [D, SK], MT, kind="ExternalInput").ap()
    queryT = nc.dram_tensor("queryT", [E, SQ], MT, kind="ExternalInput").ap()
    WkT = nc.dram_tensor("WkT", [NG, D, G * E], MT, kind="ExternalInput").ap()
    WvcT = nc.dram_tensor("WvcT", [NG, D, G * E], MT, kind="ExternalInput").ap()
    bcT = nc.dram_tensor("bcT", [E, 1], F32, kind="ExternalInput").ap()
    onesSQ = nc.dram_tensor("onesSQ", [P, P], MT, kind="ExternalInput").ap()
    out = nc.dram_tensor("out", [SQ, E], F32, kind="ExternalOutput").ap()

    from contextlib import ExitStack

    with tile.TileContext(nc) as tc, ExitStack() as es:
        constp = es.enter_context(tc.tile_pool(name="const", bufs=1))
        statesp = es.enter_context(tc.tile_pool(name="states", bufs=DCH))
        queryp = es.enter_context(tc.tile_pool(name="query", bufs=1))
        wkp = es.enter_context(tc.tile_pool(name="wk", bufs=10))
        wvp = es.enter_context(tc.tile_pool(name="wv", bufs=10))
        keysp = es.enter_context(tc.tile_pool(name="keys", bufs=2))
        expp = es.enter_context(tc.tile_pool(name="exps", bufs=8))
        valsp = es.enter_context(tc.tile_pool(name="vals", bufs=12))
        recipp = es.enter_context(tc.tile_pool(name="recip", bufs=2))
        tmpp = es.enter_context(tc.tile_pool(name="tmpn", bufs=2))
        finalp = es.enter_context(tc.tile_pool(name="final", bufs=1))
        outp = es.enter_context(tc.tile_pool(name="outs", bufs=4))
        dsump = es.enter_context(tc.tile_pool(name="dsum", bufs=12))
        ps_main = es.enter_context(tc.tile_pool(name="ps_main", bufs=4, space="PSUM"))
        ps_ctx = es.enter_context(tc.tile_pool(name="ps_ctx", bufs=4, space="PSUM"))

        # ---- constants ----
        ones_sq = constp.tile([P, P], MT)
        nc.sync.dma_start(ones_sq[:], onesSQ[:])
        ident = constp.tile([P, P], F32)
        make_identity(nc, ident[:])
        bc_t = constp.tile([E, 1], F32)
        nc.sync.dma_start(bc_t[:], bcT[:])

        # ---- resident activations ----
        st = []
        for d in range(DCH):
            st_t = statesp.tile([P, SK], MT, name="st_t")
            nc.sync.dma_start(st_t[:], statesT[d * P : (d + 1) * P, :])
            st.append(st_t)
        q_t = queryp.tile([E, SQ], MT)
        nc.sync.dma_start(q_t[:], queryT[:])

        final_t = finalp.tile([E, SQ], F32)

        def emit_norm_qh(h, pc, ssums, qh):
            """Denominator matmul + normalize one q-half of head h into final_t.

            Deferred into the NEXT head's kt loop (qh0 at kt1, qh1 at kt2) so
            the PE queue never stalls on the exp-reduction tree, and so the
            transient pd bank comes from the ps_main rotation.
            """
            pd = ps_main.tile([P, NHALF], F32, tag="m", name="pd")
            nc.tensor.matmul(
                pd[:], (ones_sq[:]), (ssums[qh][:]), start=True, stop=True
            )
            rec = recipp.tile([P, NHALF], F32, name="rec")
            nc.vector.reciprocal_approx_fast(out=rec[:], in_=pd[:])
            tmp = tmpp.tile([P, NHALF], F32, name="tmp")
            nc.vector.tensor_mul(tmp[:], pc[qh][:], rec[:])
            if h == 0:
                nc.vector.tensor_scalar(
                    final_t[:, qh * NHALF : (qh + 1) * NHALF],
                    tmp[:],
                    bc_t[:],
                    None,
                    op0=mybir.AluOpType.add,
                )
            else:
                nc.vector.tensor_add(
                    final_t[:, qh * NHALF : (qh + 1) * NHALF],
                    final_t[:, qh * NHALF : (qh + 1) * NHALF],
                    tmp[:],
                )

        pending_norm = None

        for rep in range(repeat):
            def emit_wdma(gw):
                wvt, wkt = [], []
                for d in range(DCH):
                    wv_t = wvp.tile([P, G * E], MT, name="wv_t")
                    nc.sync.dma_start(wv_t[:], WvcT[gw, d * P : (d + 1) * P, :])
                    wvt.append(wv_t)
                for d in range(DCH):
                    wk_t = wkp.tile([P, G * E], MT, name="wk_t")
                    nc.sync.dma_start(wk_t[:], WkT[gw, d * P : (d + 1) * P, :])
                    wkt.append(wk_t)
                return wvt, wkt

            def emit_vals_tile(kt, wvt):
                pv = ps_main.tile([P, G * E], F32, tag="m", name="pv")
                for d in range(DCH):
                    nc.tensor.matmul(
                        pv[:],
                        (st[d][:, kt * P : (kt + 1) * P]),
                        (wvt[d][:]),
                        start=(d == 0),
                        stop=(d == DCH - 1),
                    )
                v_sb = valsp.tile([P, G * E], MT, name="v_sb")
                nc.vector.tensor_copy(v_sb[:], pv[:])
                return v_sb

            next_w = emit_wdma(0)
            next_vals = []

            for g in range(NG):
                wv_tiles, wk_tiles = next_w
                # vals tiles not already built during the previous group's
                # last head:
                vals_tiles = next_vals
                for kt in range(len(vals_tiles), KT):
                    vals_tiles.append(emit_vals_tile(kt, wv_tiles))
                next_vals = []

                def emit_keys_block(hg_k):
                    """Full 16-matmul keys emission for head (g, hg_k)."""
                    ksb = keysp.tile([E, SK], MT, name="keys_sb")
                    for half in range(2):
                        pk = ps_main.tile(
                            [P, NHALF], F32, tag="m", name="pk"
                        )
                        for d in range(DCH):
                            nc.tensor.matmul(
                                pk[:],
                                (wk_tiles[d][:, hg_k * E : (hg_k + 1) * E]),
                                (st[d][:, half * NHALF : (half + 1) * NHALF]),
                                start=(d == 0),
                                stop=(d == DCH - 1),
                            )
                        nc.vector.tensor_copy(
                            ksb[:, half * NHALF : (half + 1) * NHALF], pk[:]
                        )
                    return ksb

                # first head's keys as one block (nothing to hide them under)
                next_keys_sb = emit_keys_block(0)

                for hg in range(G):
                    h = g * G + hg
                    keys_sb = next_keys_sb

                    # next head's keys are interleaved into this head's kt
                    # loop (2 matmuls per kt) to keep PE saturated while the
                    # softmax phase is ACT-paced. kts 0-3 build half 0,
                    # kts 4-7 half 1.
                    ik_hg = hg + 1 if hg + 1 < G else None
                    if ik_hg is not None:
                        next_keys_sb = keysp.tile([E, SK], MT, name="keys_sb")
                        ik_pk = None
                    iv_w = None
                    if ik_hg is None and g + 1 < NG:
                        next_w = emit_wdma(g + 1)
                        iv_w = next_w[0]
                        iv_pv = None

                    # ---- scores -> exp -> denom/ctx accumulate, per (kt, qh) ----
                    pc = [
                        ps_ctx.tile(
                            [P, NHALF], F32, tag="ctx", name="pc"
                        )
                        for i in range(2)
                    ]
                    # Software-pipelined: emit kt's scores+exp, then kt-1's
                    # ctx matmuls (so PE never waits on the just-issued exp).
                    # Denominator: gpsimd pair-adds of exp tiles as pairs
                    # complete (Pool engine is otherwise idle), DVE finishes
                    # the reduction tree, then ONE ones-matmul per half.
                    pending = None  # ([ex_qh0, ex_qh1], kt)
                    last_ex = [None, None]
                    partials = [[], []]
                    for kt in range(KT):
                        if kt in (1, 2) and pending_norm is not None:
                            # previous head's denominator matmul + normalize,
                            # deferred here so PE never stalls on its tree.
                            emit_norm_qh(*pending_norm, kt - 1)
                            if kt == 2:
                                pending_norm = None
                        exs = []
                        for qh in range(2):
                            ps = ps_main.tile(
                                [P, NHALF], F32, tag="m",
                                name="ps",
                            )
                            nc.tensor.matmul(
                                ps[:],
                                (keys_sb[:, kt * P : (kt + 1) * P]),
                                (q_t[:, qh * NHALF : (qh + 1) * NHALF]),
                                start=True,
                                stop=True,
                            )
                            ex = expp.tile(
                                [P, NHALF], MT, name="ex"
                            )
                            nc.scalar.activation(
                                ex[:], ps[:], mybir.ActivationFunctionType.Exp,
                                scale=SCALE,
                            )
                            exs.append(ex)
                            if kt % 2 == 1:
                                gsum = dsump.tile([P, NHALF], MT, name="gsum")
                                nc.vector.tensor_add(
                                    gsum[:], last_ex[qh][:], ex[:]
                                )
                                partials[qh].append(gsum)
                            else:
                                last_ex[qh] = ex
                        if iv_w is not None:
                            # 2 vals matmuls of group g+1 per kt: tile kt//4,
                            # d-chunks 2*(kt%4) and 2*(kt%4)+1
                            vkt = kt // 4
                            loc = kt % 4
                            if loc == 0:
                                iv_pv = ps_main.tile(
                                    [P, G * E], F32, tag="m", name="pv"
                                )
                            for d in (2 * loc, 2 * loc + 1):
                                nc.tensor.matmul(
                                    iv_pv[:],
                                    (st[d][:, vkt * P : (vkt + 1) * P]),
                                    (iv_w[d][:]),
                                    start=(d == 0),
                                    stop=(d == DCH - 1),
                                )
                            if loc == 3:
                                v_sb = valsp.tile([P, G * E], MT, name="v_sb")
                                nc.vector.tensor_copy(v_sb[:], iv_pv[:])
                                next_vals.append(v_sb)
                        if ik_hg is not None:
                            half = kt // 4
                            loc = kt % 4
                            if loc == 0:
                                ik_pk = ps_main.tile(
                                    [P, NHALF], F32, tag="m", name="pk"
                                )
                            for d in (2 * loc, 2 * loc + 1):
                                nc.tensor.matmul(
                                    ik_pk[:],
                                    (wk_tiles[d][:, ik_hg * E : (ik_hg + 1) * E]),
                                    (st[d][:, half * NHALF : (half + 1) * NHALF]),
                                    start=(d == 0),
                                    stop=(d == DCH - 1),
                                )
                            if loc == 3:
                                nc.vector.tensor_copy(
                                    next_keys_sb[
                                        :, half * NHALF : (half + 1) * NHALF
                                    ],
                                    ik_pk[:],
                                )
                        if pending is not None:
                            pexs, pkt = pending
                            for qh in range(2):
                                nc.tensor.matmul(
                                    pc[qh][:],
                                    (vals_tiles[pkt][:, hg * E : (hg + 1) * E]),
                                    (pexs[qh][:]),
                                    start=(pkt == 0),
                                    stop=False,
                                )
                        pending = (exs, kt)
                    pexs, pkt = pending
                    for qh in range(2):
                        nc.tensor.matmul(
                            pc[qh][:],
                            (vals_tiles[pkt][:, hg * E : (hg + 1) * E]),
                            (pexs[qh][:]),
                            start=False,
                            stop=True,
                        )
                    ssums = []
                    for qh in range(2):
                        p0, p1, p2, p3 = partials[qh]
                        d1 = dsump.tile([P, NHALF], MT, name="gsum")
                        nc.vector.tensor_add(d1[:], p0[:], p1[:])
                        d2 = dsump.tile([P, NHALF], MT, name="gsum")
                        nc.vector.tensor_add(d2[:], p2[:], p3[:])
                        ssum = dsump.tile([P, NHALF], MT, name="gsum")
                        nc.vector.tensor_add(ssum[:], d1[:], d2[:])
                        ssums.append(ssum)

                    # defer denom matmuls + normalize into next head's kt loop
                    pending_norm = (h, pc, ssums)

            emit_norm_qh(*pending_norm, 0)
            emit_norm_qh(*pending_norm, 1)
            pending_norm = None

            # ---- transpose finalT -> out [SQ, E] ----
            for qt in range(KT):
                pt = ps_main.tile([P, P], F32, tag="m", name="pt")
                nc.tensor.transpose(
                    pt[:], final_t[:, qt * P : (qt + 1) * P], ident[:]
                )
                o_sb = outp.tile([P, E], F32, name="o_sb")
                nc.vector.tensor_copy(o_sb[:], pt[:])
                nc.sync.dma_start(out[qt * P : (qt + 1) * P, :], o_sb[:])

    nc.compile()
    return nc


def _prep_inputs(query, states, Wk, bk, Wv, bv, Wc, bc):
    """Host-side sharding: per-core input maps (core c == batch element c).

    bk is dropped (softmax shift invariance); bv is folded into bc; Wc is
    folded into Wv (Wvc_h = Wv_h @ Wc_h):
    out = sum_h attn_h @ (states @ Wvc_h) + (bc + bv.flatten() @ Wc).
    """
    query = np.asarray(query, np.float32)
    states = np.asarray(states, np.float32)
    Wk = np.asarray(Wk, np.float64)
    Wv = np.asarray(Wv, np.float64)
    Wc = np.asarray(Wc, np.float64)
    bv = np.asarray(bv, np.float64)
    bc = np.asarray(bc, np.float64)

    # Wvc[h] = Wv[h] @ Wc[h*E:(h+1)*E, :]  -> [H, D, E]
    Wvc = np.einsum("hde,hef->hdf", Wv, Wc.reshape(H, E, E))

    WkT = np.ascontiguousarray(
        Wk.transpose(1, 0, 2).reshape(D, NG, G * E).transpose(1, 0, 2)
    ).astype(ml_dtypes.bfloat16)
    WvcT = np.ascontiguousarray(
        Wvc.transpose(1, 0, 2).reshape(D, NG, G * E).transpose(1, 0, 2)
    ).astype(ml_dtypes.bfloat16)
    bc_eff = (bc + bv.reshape(H * E) @ Wc).astype(np.float32)
    bcT = np.ascontiguousarray(bc_eff.reshape(E, 1))

    in_maps = []
    for c in range(N_CORES):
        in_maps.append(
            {
                "statesT": np.ascontiguousarray(states[c].T).astype(ml_dtypes.bfloat16),
                "queryT": np.ascontiguousarray(query[c].T).astype(ml_dtypes.bfloat16),
                "WkT": WkT,
                "WvcT": WvcT,
                "bcT": bcT,
                "onesSQ": _ONES_SQ,
            }
        )
    return in_maps


def get_nc(mm_dtype="bf16", repeat=1):
    key = (mm_dtype, repeat)
    nc = _COMPILED.get(key)
    if nc is None:
        nc = build_nc(mm_dtype, repeat=repeat)
        _COMPILED[key] = nc
    return nc


def kernel(query, states, Wk, bk, Wv, bv, Wc, bc):
    nc = get_nc()
    in_maps = _prep_inputs(query, states, Wk, bk, Wv, bv, Wc, bc)
    res = bass_utils.run_bass_kernel_spmd(nc, in_maps, list(range(N_CORES)))
    return np.stack([res.results[c]["out"] for c in range(N_CORES)], axis=0)
